# revision 11
# baseline (speedup 1.0000x reference)
"""AttentionBlock (GroupNorm + single-head-per-core spatial attention + proj)
for Trainium2, 8 NeuronCores.

Sharding: core i handles (batch b = i//4, head h = i%4).  Each core computes
its head's attention output projected through its slice of proj_w's input
channels; the host sums the 4 per-head partials per batch (tensor-parallel
unshard) and adds residual + biases.

Reference semantics (B=2, C=128, H=W=64, heads=4, groups=32, eps=1e-5):
  h   = groupnorm(x) * nw + nb
  qkv = qkv_w @ h + qkv_b          (1x1 conv == channel matmul)
  S   = (q^T k) / sqrt(32); A = softmax(S, axis=t); out = v A^T
  y   = proj_w @ out + proj_b + x

Device-side structure:
  - groupnorm folded into qkv weights: q = (Wq*scale_c)^T X + (bq + Wq^T shift_c)
  - q,k produced 4x-replicated across partition quadrants (via 4x-tiled
    weights) so the S^T matmuls can be packed 4-up with tile_position row
    tiling: K=4x32=128 active PE rows.  K=32 matmuls never trip the HAM
    busy detector and leave the PE throttled at 1.2 GHz.
  - v computed directly transposed (v^T[t,d]) from X with a ones column
    appended -> AV matmul yields [denom; U] rows in one pass
  - softmax has no max-subtraction (|S/sqrt(d)| <= ~7 for this distribution)
  - exp on ScalarE with the 1/sqrt(d) scale folded into the activation affine
  - proj lhsT gets an extra row (addvec) so that PP = proj(U) + addvec x denom,
    and PP * (1/denom) = proj(attn_out) including the groupnorm-shift v term
  - residual, proj_b, and the v-bias term (proj_w @ bv) are added on host
"""

import sys

sys.path.insert(0, "/opt/trn_rl_repo")

import numpy as np

import concourse.bass as bass
import concourse.tile as tile
from concourse import bacc, mybir
from concourse.bass_utils import run_bass_kernel_spmd

F32 = mybir.dt.float32
F32R = mybir.dt.float32r

B, C, HW = 2, 128, 4096
NH, DH = 4, 32
NG, GS = 32, 4  # groups, channels per group
EPS = 1e-5
SCALE = 1.0 / np.sqrt(DH)

N_CORES = 8


def build_program():
    nc = bacc.Bacc("TRN2", target_bir_lowering=False, debug=False)

    def din(name, shape):
        return nc.dram_tensor(name, shape, F32, kind="ExternalInput").ap()

    xs = din("xs", [C, HW])
    cpk = din("cpk", [C, 324])  # packed: wqT4|wkT4|wvT|bq4|bk4|gs|nw|nb
    pwTa = din("pwTa", [DH, C])
    g2 = din("g2", [NG, C])
    out_d = nc.dram_tensor("out_p", [C, HW], F32, kind="ExternalOutput").ap()

    with tile.TileContext(nc) as tc:
        with (
            tc.tile_pool(name="consts", bufs=1) as consts,
            tc.tile_pool(name="xpool", bufs=1) as xpool,
            tc.tile_pool(name="qk", bufs=1) as qkpool,
            tc.tile_pool(name="vt", bufs=1) as vtpool,
            tc.tile_pool(name="small", bufs=8) as small,
            tc.tile_pool(name="epool", bufs=3) as epool,
            tc.tile_pool(name="upool", bufs=2) as upool,
            tc.tile_pool(name="bcpool", bufs=2) as bcpool,
            tc.tile_pool(name="opool", bufs=2) as opool,
        ):
            # ---------------- load inputs ----------------
            # X split across 4 DMA queues (different engines) so chunks land
            # in parallel and bn_stats/Xr-cast can start on early chunks.
            X = xpool.tile([C, HW], F32)
            dma_engs = [nc.sync, nc.scalar, nc.gpsimd, nc.sync]
            for j in range(4):
                dma_engs[j].dma_start(
                    X[:, 1024 * j : 1024 * (j + 1)], xs[:, 1024 * j : 1024 * (j + 1)]
                )
            # packed per-partition consts: one DMA
            cbuf = consts.tile([C, 324], F32)
            nc.sync.dma_start(cbuf[:], cpk[:])
            c_wqT4 = cbuf[:, 0:128]
            c_wkT4 = cbuf[:, 128:256]
            c_wvT = cbuf[:, 256:288]
            c_bq4 = cbuf[:, 288:289]
            c_bk4 = cbuf[:, 289:290]
            c_gs = cbuf[:, 290:322]
            c_nw = cbuf[:, 322:323]
            c_nb = cbuf[:, 323:324]
            c_pwTa = consts.tile([DH, C], F32)
            nc.sync.dma_start(c_pwTa[:], pwTa[:])
            c_g2 = consts.tile([NG, C], F32)
            nc.sync.dma_start(c_g2[:], g2[:])
            # proj lhsT: row 0 = runtime addvec, rows 1..32 = pwTa
            pwaug = consts.tile([DH + 1, C], F32)
            nc.sync.dma_start(pwaug[1 : DH + 1, :], pwTa[:])
            eps_t = consts.tile([NG, 1], F32)
            nc.vector.memset(eps_t[:], EPS)
            # f32r copy of X (on otherwise-idle ScalarE) for fast qkv matmuls
            Xr = xpool.tile([C, HW], F32R, tag="Xr")
            for j in range(4):
                nc.scalar.copy(
                    out=Xr[:, 1024 * j : 1024 * (j + 1)],
                    in_=X[:, 1024 * j : 1024 * (j + 1)],
                )

            with (
                tc.tile_pool(name="pp", bufs=2, space="PSUM") as pp,
                tc.tile_pool(name="sqpool", bufs=2, space="PSUM") as sqpool,
                tc.tile_pool(name="accp", bufs=1, space="PSUM") as accp,
                tc.tile_pool(name="projp", bufs=1, space="PSUM") as projp,
            ):
                # ---------------- groupnorm stats ----------------
                Xg = X[:].rearrange("c (n f) -> c n f", f=512)
                stats = small.tile([C, 8, 6], F32)
                for i in range(8):
                    nc.vector.bn_stats(out=stats[:, i, :], in_=Xg[:, i, :])
                mv = small.tile([C, 2], F32)
                nc.vector.bn_aggr(out=mv[:], in_=stats[:])
                # mv2 = [mean_c, E[x^2]_c]
                mv2 = small.tile([C, 2], F32)
                nc.vector.tensor_copy(out=mv2[:, 0:1], in_=mv[:, 0:1])
                nc.vector.tensor_tensor(
                    out=mv2[:, 1:2], in0=mv[:, 0:1], in1=mv[:, 0:1],
                    op=mybir.AluOpType.mult,
                )
                nc.vector.tensor_tensor(
                    out=mv2[:, 1:2], in0=mv2[:, 1:2], in1=mv[:, 1:2],
                    op=mybir.AluOpType.add,
                )
                # per-group [m_g, E_g[x^2]]
                gstat_ps = pp.tile([NG, 2], F32, tag="pp")
                nc.tensor.matmul(gstat_ps[:], lhsT=c_gs[:], rhs=mv2[:])
                gstat = small.tile([NG, 2], F32)
                nc.vector.tensor_copy(out=gstat[:], in_=gstat_ps[:])
                # var_g = E[x^2] - m^2 ; rstd = 1/sqrt(var+eps)
                varg = small.tile([NG, 1], F32)
                nc.vector.tensor_tensor(
                    out=varg[:], in0=gstat[:, 0:1], in1=gstat[:, 0:1],
                    op=mybir.AluOpType.mult,
                )
                nc.vector.tensor_tensor(
                    out=varg[:], in0=gstat[:, 1:2], in1=varg[:],
                    op=mybir.AluOpType.subtract,
                )
                stdg = small.tile([NG, 1], F32)
                nc.scalar.activation(
                    out=stdg[:], in_=varg[:],
                    func=mybir.ActivationFunctionType.Sqrt,
                    bias=eps_t[:], scale=1.0,
                )
                rstdg = small.tile([NG, 1], F32)
                nc.vector.reciprocal(out=rstdg[:], in_=stdg[:])
                gexp = small.tile([NG, 2], F32)
                nc.vector.tensor_copy(out=gexp[:, 0:1], in_=gstat[:, 0:1])
                nc.vector.tensor_copy(out=gexp[:, 1:2], in_=rstdg[:])
                # expand to channels
                mrc_ps = pp.tile([C, 2], F32, tag="pp")
                nc.tensor.matmul(mrc_ps[:], lhsT=c_g2[:], rhs=gexp[:])
                mrc = small.tile([C, 2], F32)
                nc.vector.tensor_copy(out=mrc[:], in_=mrc_ps[:])
                # scale_c = rstd_c * nw ; shift_c = nb - mean_c*scale_c
                scale_c = small.tile([C, 1], F32)
                nc.vector.tensor_tensor(
                    out=scale_c[:], in0=mrc[:, 1:2], in1=c_nw[:],
                    op=mybir.AluOpType.mult,
                )
                shift_c = small.tile([C, 1], F32)
                nc.vector.tensor_tensor(
                    out=shift_c[:], in0=mrc[:, 0:1], in1=scale_c[:],
                    op=mybir.AluOpType.mult,
                )
                nc.vector.tensor_tensor(
                    out=shift_c[:], in0=c_nb[:], in1=shift_c[:],
                    op=mybir.AluOpType.subtract,
                )
                # folded weights
                wq_f = consts.tile([C, 128], F32R)
                nc.vector.tensor_scalar_mul(out=wq_f[:], in0=c_wqT4[:], scalar1=scale_c[:])
                wk_f = consts.tile([C, 128], F32R)
                nc.vector.tensor_scalar_mul(out=wk_f[:], in0=c_wkT4[:], scalar1=scale_c[:])
                wv_f = consts.tile([C, DH], F32R)
                nc.vector.tensor_scalar_mul(out=wv_f[:], in0=c_wvT[:], scalar1=scale_c[:])
                # adjusted q/k biases (4x-replicated) ; v shift
                bq_ps = pp.tile([128, 1], F32, tag="pp")
                nc.tensor.matmul(bq_ps[:], lhsT=c_wqT4[:], rhs=shift_c[:])
                bq_f = small.tile([128, 1], F32)
                nc.vector.tensor_tensor(
                    out=bq_f[:], in0=bq_ps[:], in1=c_bq4[:], op=mybir.AluOpType.add
                )
                bk_ps = pp.tile([128, 1], F32, tag="pp")
                nc.tensor.matmul(bk_ps[:], lhsT=c_wkT4[:], rhs=shift_c[:])
                bk_f = small.tile([128, 1], F32)
                nc.vector.tensor_tensor(
                    out=bk_f[:], in0=bk_ps[:], in1=c_bk4[:], op=mybir.AluOpType.add
                )
                vs_ps = pp.tile([DH, 1], F32, tag="pp")
                nc.tensor.matmul(vs_ps[:], lhsT=c_wvT[:], rhs=shift_c[:])
                vsum = small.tile([DH, 1], F32)
                nc.vector.tensor_copy(out=vsum[:], in_=vs_ps[:])
                # addvec row: pwaug[0, :] = vsum^T pwTa
                av_ps = pp.tile([1, C], F32, tag="pp")
                nc.tensor.matmul(av_ps[:], lhsT=vsum[:], rhs=c_pwTa[:])
                nc.vector.tensor_copy(out=pwaug[0:1, :], in_=av_ps[:])

                # v^T ones column (tile written chunk-wise inside sc==0 loop)
                v_t = vtpool.tile([128, 32, DH + 1], F32R)
                ones_f = consts.tile([128, 1], F32)
                nc.vector.memset(ones_f[:], 1.0)
                nc.vector.tensor_copy(
                    out=v_t[:, :, 0], in_=ones_f[:, 0:1].to_broadcast([128, 32])
                )
                q_sb = qkpool.tile([128, HW], F32R, tag="q")
                k_sb = qkpool.tile([128, HW], F32R, tag="k")

                # ---------------- attention + epilogue ----------------
                # per s-chunk (512 cols): 8 quads of 4 t-blocks; S^T
                # row-packed 4-up (K=128 active rows -> HAM stays warm);
                # exp in [128,1024] ops; AV accumulates [denom; U] in one
                # PSUM bank.  q/k/v_t production is interleaved into the
                # loops right before first use so the exp stream starts as
                # early as possible and the builds hide in PE/ACT slack.
                ident = mybir.ActivationFunctionType.Identity
                for sc in range(HW // 512):
                    so = 512 * sc
                    qp = pp.tile([128, 512], F32, tag="pp")
                    nc.tensor.matmul(qp[:], lhsT=wq_f[:], rhs=Xr[:, so : so + 512])
                    nc.scalar.activation(
                        out=q_sb[:, so : so + 512], in_=qp[:], func=ident,
                        bias=bq_f[:], scale=1.0,
                    )
                    acc = accp.tile([DH + 1, 512], F32)
                    prev = None
                    for tq in range(8):
                        if sc == 0:
                            kp = pp.tile([128, 512], F32, tag="pp")
                            nc.tensor.matmul(
                                kp[:], lhsT=wk_f[:],
                                rhs=Xr[:, 512 * tq : 512 * (tq + 1)],
                            )
                            nc.scalar.activation(
                                out=k_sb[:, 512 * tq : 512 * (tq + 1)], in_=kp[:],
                                func=ident, bias=bk_f[:], scale=1.0,
                            )
                        SA = sqpool.tile([128, 1024], F32, tag="sq")
                        SB = sqpool.tile([128, 1024], F32, tag="sq")
                        for i in range(4):
                            dst = (SA if i < 2 else SB)[:, 512 * (i % 2) : 512 * (i % 2 + 1)]
                            nc.tensor.matmul(
                                dst,
                                lhsT=k_sb[32 * i : 32 * (i + 1),
                                          128 * (4 * tq + i) : 128 * (4 * tq + i + 1)],
                                rhs=q_sb[32 * i : 32 * (i + 1), so : so + 512],
                                tile_position=(32 * i, 0),
                            )
                        if sc == 0:
                            for i in range(4):
                                tb = 4 * tq + i
                                vp = pp.tile([128, DH], F32, tag="pp")
                                nc.tensor.matmul(
                                    vp[:], lhsT=Xr[:, 128 * tb : 128 * (tb + 1)],
                                    rhs=wv_f[:],
                                )
                                nc.vector.tensor_copy(
                                    out=v_t[:, tb, 1 : DH + 1], in_=vp[:]
                                )
                        if prev is not None:
                            pA, pB, ptq = prev
                            for i in range(4):
                                src = (pA if i < 2 else pB)[:, 512 * (i % 2) : 512 * (i % 2 + 1)]
                                nc.tensor.matmul(
                                    acc[:],
                                    lhsT=v_t[:, 4 * ptq + i, :],
                                    rhs=src,
                                    start=(ptq == 0 and i == 0),
                                    stop=False,
                                )
                        EA = epool.tile([128, 1024], F32R, tag="E")
                        nc.scalar.activation(
                            out=EA[:], in_=SA[:],
                            func=mybir.ActivationFunctionType.Exp,
                            scale=float(SCALE),
                        )
                        EB = epool.tile([128, 1024], F32R, tag="E")
                        nc.scalar.activation(
                            out=EB[:], in_=SB[:],
                            func=mybir.ActivationFunctionType.Exp,
                            scale=float(SCALE),
                        )
                        prev = (EA, EB, tq)
                    pA, pB, ptq = prev
                    for i in range(4):
                        src = (pA if i < 2 else pB)[:, 512 * (i % 2) : 512 * (i % 2 + 1)]
                        nc.tensor.matmul(
                            acc[:],
                            lhsT=v_t[:, 4 * ptq + i, :],
                            rhs=src,
                            start=False,
                            stop=(i == 3),
                        )

                    # epilogue: U -> proj -> normalize -> store
                    U = upool.tile([DH + 1, 512], F32)
                    nc.vector.tensor_copy(out=U[:], in_=acc[:])
                    recip = upool.tile([1, 512], F32, tag="recip")
                    nc.vector.reciprocal(out=recip[:], in_=U[0:1, :])
                    bc = bcpool.tile([C, 512], F32)
                    nc.gpsimd.partition_broadcast(bc[:], recip[:], channels=C)
                    pj = projp.tile([C, 512], F32)
                    nc.tensor.matmul(pj[:], lhsT=pwaug[:], rhs=U[:])
                    out_sb = opool.tile([C, 512], F32)
                    nc.vector.tensor_tensor(
                        out=out_sb[:], in0=pj[:], in1=bc[:], op=mybir.AluOpType.mult
                    )
                    nc.sync.dma_start(out=out_d[:, so : so + 512], in_=out_sb[:])

    nc.compile()
    return nc


_NC_CACHE = None


def _get_program():
    global _NC_CACHE
    if _NC_CACHE is None:
        _NC_CACHE = build_program()
    return _NC_CACHE


def kernel(x, norm_w, norm_b, qkv_w, qkv_b, proj_w, proj_b):
    x = np.asarray(x, np.float32)
    norm_w = np.asarray(norm_w, np.float32)
    norm_b = np.asarray(norm_b, np.float32)
    qkv_w = np.asarray(qkv_w, np.float32)
    qkv_b = np.asarray(qkv_b, np.float32)
    proj_w = np.asarray(proj_w, np.float32)
    proj_b = np.asarray(proj_b, np.float32)

    nc = _get_program()

    gs = np.zeros((C, NG), np.float32)
    gs[np.arange(C), np.arange(C) // GS] = 1.0 / GS
    g2 = np.zeros((NG, C), np.float32)
    g2[np.arange(C) // GS, np.arange(C)] = 1.0

    in_maps = []
    for ci in range(N_CORES):
        b, h = ci // NH, ci % NH
        sl = slice(DH * h, DH * (h + 1))
        wqT = qkv_w[sl, :].T
        wkT = qkv_w[C:][sl, :].T
        cpk = np.concatenate(
            [
                np.tile(wqT, (1, 4)),
                np.tile(wkT, (1, 4)),
                qkv_w[2 * C:][sl, :].T,
                np.tile(qkv_b[sl].reshape(DH, 1), (4, 1)),
                np.tile(qkv_b[C:][sl].reshape(DH, 1), (4, 1)),
                gs,
                norm_w.reshape(C, 1),
                norm_b.reshape(C, 1),
            ],
            axis=1,
        )
        in_maps.append(
            {
                "xs": np.ascontiguousarray(x[b].reshape(C, HW)),
                "cpk": np.ascontiguousarray(cpk),
                "pwTa": np.ascontiguousarray(proj_w[:, sl].T),
                "g2": g2,
            }
        )

    res = run_bass_kernel_spmd(nc, in_maps, core_ids=list(range(N_CORES)))

    # unshard: sum per-head partials, add residual + proj bias + v-bias term
    base = proj_b + proj_w @ qkv_b[2 * C :]
    out = np.empty((B, C, HW), np.float32)
    for b in range(B):
        acc = np.zeros((C, HW), np.float32)
        for h in range(NH):
            acc += res.results[b * NH + h]["out_p"]
        out[b] = acc + x[b].reshape(C, HW) + base[:, None]
    return out.reshape(B, C, 64, 64)


# revision 12
# speedup vs baseline: 1.2003x; 1.2003x over previous
"""AttentionBlock (GroupNorm + single-head-per-core spatial attention + proj)
for Trainium2, 8 NeuronCores.

Sharding: core i handles (batch b = i//4, head h = i%4).  Each core computes
its head's attention output projected through its slice of proj_w's input
channels; the host sums the 4 per-head partials per batch (tensor-parallel
unshard) and adds residual + biases.

Reference semantics (B=2, C=128, H=W=64, heads=4, groups=32, eps=1e-5):
  h   = groupnorm(x) * nw + nb
  qkv = qkv_w @ h + qkv_b          (1x1 conv == channel matmul)
  S   = (q^T k) / sqrt(32); A = softmax(S, axis=t); out = v A^T
  y   = proj_w @ out + proj_b + x

Device-side structure:
  - groupnorm folded into qkv weights: q = (Wq*scale_c)^T X + (bq + Wq^T shift_c)
  - q,k produced 4x-replicated across partition quadrants (via 4x-tiled
    weights) so the S^T matmuls can be packed 4-up with tile_position row
    tiling: K=4x32=128 active PE rows.  K=32 matmuls never trip the HAM
    busy detector and leave the PE throttled at 1.2 GHz.
  - v computed directly transposed (v^T[t,d]) from X with a ones column
    appended -> AV matmul yields [denom; U] rows in one pass
  - softmax has no max-subtraction (|S/sqrt(d)| <= ~7 for this distribution)
  - exp on ScalarE with the 1/sqrt(d) scale folded into the activation affine
  - proj lhsT gets an extra row (addvec) so that PP = proj(U) + addvec x denom,
    and PP * (1/denom) = proj(attn_out) including the groupnorm-shift v term
  - residual, proj_b, and the v-bias term (proj_w @ bv) are added on host
"""

import sys

sys.path.insert(0, "/opt/trn_rl_repo")

import numpy as np

import concourse.bass as bass
import concourse.tile as tile
from concourse import bacc, mybir
from concourse.bass_utils import run_bass_kernel_spmd

F32 = mybir.dt.float32
F32R = mybir.dt.float32r

B, C, HW = 2, 128, 4096
NH, DH = 4, 32
NG, GS = 32, 4  # groups, channels per group
EPS = 1e-5
SCALE = 1.0 / np.sqrt(DH)

N_CORES = 8


def build_program():
    nc = bacc.Bacc("TRN2", target_bir_lowering=False, debug=False)

    def din(name, shape):
        return nc.dram_tensor(name, shape, F32, kind="ExternalInput").ap()

    xs = din("xs", [C, HW])
    cpk = din("cpk", [C, 324])  # packed: wqT4|wkT4|wvT|bq4|bk4|gs|nw|nb
    pwTa = din("pwTa", [DH, C])
    g2 = din("g2", [NG, C])
    out_d = nc.dram_tensor("out_p", [C, HW], F32, kind="ExternalOutput").ap()

    with tile.TileContext(nc) as tc:
        with (
            tc.tile_pool(name="consts", bufs=1) as consts,
            tc.tile_pool(name="xpool", bufs=1) as xpool,
            tc.tile_pool(name="qk", bufs=1) as qkpool,
            tc.tile_pool(name="vt", bufs=1) as vtpool,
            tc.tile_pool(name="small", bufs=8) as small,
            tc.tile_pool(name="epool", bufs=3) as epool,
            tc.tile_pool(name="upool", bufs=2) as upool,
            tc.tile_pool(name="bcpool", bufs=2) as bcpool,
            tc.tile_pool(name="opool", bufs=2) as opool,
        ):
            # ---------------- load inputs ----------------
            # X split across 4 DMA queues (different engines) so chunks land
            # in parallel and bn_stats/Xr-cast can start on early chunks.
            X = xpool.tile([C, HW], F32)
            dma_engs = [nc.sync, nc.scalar, nc.gpsimd, nc.sync]
            for j in range(4):
                dma_engs[j].dma_start(
                    X[:, 1024 * j : 1024 * (j + 1)], xs[:, 1024 * j : 1024 * (j + 1)]
                )
            # packed per-partition consts: one DMA
            cbuf = consts.tile([C, 324], F32)
            nc.sync.dma_start(cbuf[:], cpk[:])
            c_wqT4 = cbuf[:, 0:128]
            c_wkT4 = cbuf[:, 128:256]
            c_wvT = cbuf[:, 256:288]
            c_bq4 = cbuf[:, 288:289]
            c_bk4 = cbuf[:, 289:290]
            c_gs = cbuf[:, 290:322]
            c_nw = cbuf[:, 322:323]
            c_nb = cbuf[:, 323:324]
            c_pwTa = consts.tile([DH, C], F32)
            nc.sync.dma_start(c_pwTa[:], pwTa[:])
            c_g2 = consts.tile([NG, C], F32)
            nc.sync.dma_start(c_g2[:], g2[:])
            # proj lhsT: row 0 = runtime addvec, rows 1..32 = pwTa
            pwaug = consts.tile([DH + 1, C], F32)
            nc.sync.dma_start(pwaug[1 : DH + 1, :], pwTa[:])
            eps_t = consts.tile([NG, 1], F32)
            nc.vector.memset(eps_t[:], EPS)
            # f32r copy of X (on otherwise-idle ScalarE) for fast qkv matmuls
            Xr = xpool.tile([C, HW], F32R, tag="Xr")
            for j in range(4):
                nc.scalar.copy(
                    out=Xr[:, 1024 * j : 1024 * (j + 1)],
                    in_=X[:, 1024 * j : 1024 * (j + 1)],
                )

            # PE warmup: junk f32r matmuls with no upstream deps keep the
            # HAM activity monitor busy during the X-load + stats window so
            # the attention stream starts at 2.4 GHz instead of 1.2.
            scr = qkpool.tile([128, 512], F32R, tag="scratch")
            scr_o = qkpool.tile([128, 4], F32, tag="scr_o")
            ones_w = consts.tile([128, 1], F32)
            nc.vector.memset(ones_w[:], 1.0)
            nc.vector.tensor_copy(
                out=scr[:], in_=ones_w[:, 0:1].to_broadcast([128, 512])
            )
            with tc.tile_pool(name="warm", bufs=1, space="PSUM") as warm:
                wp = warm.tile([128, 512], F32)
                for wi in range(24):
                    nc.tensor.matmul(
                        wp[:], lhsT=scr[:, 0:128], rhs=scr[:],
                        start=(wi == 0), stop=(wi == 23),
                    )
                nc.vector.tensor_copy(out=scr_o[:], in_=wp[:, 0:4])

            with tc.tile_pool(name="pp", bufs=3, space="PSUM") as pp:
                # ---------------- groupnorm stats ----------------
                Xg = X[:].rearrange("c (n f) -> c n f", f=512)
                stats = small.tile([C, 8, 6], F32)
                for i in range(8):
                    nc.vector.bn_stats(out=stats[:, i, :], in_=Xg[:, i, :])
                mv = small.tile([C, 2], F32)
                nc.vector.bn_aggr(out=mv[:], in_=stats[:])
                # mv2 = [mean_c, E[x^2]_c]
                mv2 = small.tile([C, 2], F32)
                nc.vector.tensor_copy(out=mv2[:, 0:1], in_=mv[:, 0:1])
                nc.vector.tensor_tensor(
                    out=mv2[:, 1:2], in0=mv[:, 0:1], in1=mv[:, 0:1],
                    op=mybir.AluOpType.mult,
                )
                nc.vector.tensor_tensor(
                    out=mv2[:, 1:2], in0=mv2[:, 1:2], in1=mv[:, 1:2],
                    op=mybir.AluOpType.add,
                )
                # per-group [m_g, E_g[x^2]]
                gstat_ps = pp.tile([NG, 2], F32, tag="pp")
                nc.tensor.matmul(gstat_ps[:], lhsT=c_gs[:], rhs=mv2[:])
                gstat = small.tile([NG, 2], F32)
                nc.vector.tensor_copy(out=gstat[:], in_=gstat_ps[:])
                # var_g = E[x^2] - m^2 ; rstd = 1/sqrt(var+eps)
                varg = small.tile([NG, 1], F32)
                nc.vector.tensor_tensor(
                    out=varg[:], in0=gstat[:, 0:1], in1=gstat[:, 0:1],
                    op=mybir.AluOpType.mult,
                )
                nc.vector.tensor_tensor(
                    out=varg[:], in0=gstat[:, 1:2], in1=varg[:],
                    op=mybir.AluOpType.subtract,
                )
                stdg = small.tile([NG, 1], F32)
                nc.scalar.activation(
                    out=stdg[:], in_=varg[:],
                    func=mybir.ActivationFunctionType.Sqrt,
                    bias=eps_t[:], scale=1.0,
                )
                rstdg = small.tile([NG, 1], F32)
                nc.vector.reciprocal(out=rstdg[:], in_=stdg[:])
                gexp = small.tile([NG, 2], F32)
                nc.vector.tensor_copy(out=gexp[:, 0:1], in_=gstat[:, 0:1])
                nc.vector.tensor_copy(out=gexp[:, 1:2], in_=rstdg[:])
                # expand to channels
                mrc_ps = pp.tile([C, 2], F32, tag="pp")
                nc.tensor.matmul(mrc_ps[:], lhsT=c_g2[:], rhs=gexp[:])
                mrc = small.tile([C, 2], F32)
                nc.vector.tensor_copy(out=mrc[:], in_=mrc_ps[:])
                # scale_c = rstd_c * nw ; shift_c = nb - mean_c*scale_c
                scale_c = small.tile([C, 1], F32)
                nc.vector.tensor_tensor(
                    out=scale_c[:], in0=mrc[:, 1:2], in1=c_nw[:],
                    op=mybir.AluOpType.mult,
                )
                shift_c = small.tile([C, 1], F32)
                nc.vector.tensor_tensor(
                    out=shift_c[:], in0=mrc[:, 0:1], in1=scale_c[:],
                    op=mybir.AluOpType.mult,
                )
                nc.vector.tensor_tensor(
                    out=shift_c[:], in0=c_nb[:], in1=shift_c[:],
                    op=mybir.AluOpType.subtract,
                )
                # folded weights
                wq_f = consts.tile([C, 128], F32R)
                nc.vector.tensor_scalar_mul(out=wq_f[:], in0=c_wqT4[:], scalar1=scale_c[:])
                wk_f = consts.tile([C, 128], F32R)
                nc.vector.tensor_scalar_mul(out=wk_f[:], in0=c_wkT4[:], scalar1=scale_c[:])
                wv_f = consts.tile([C, DH], F32R)
                nc.vector.tensor_scalar_mul(out=wv_f[:], in0=c_wvT[:], scalar1=scale_c[:])
                # adjusted q/k biases (4x-replicated) ; v shift
                bq_ps = pp.tile([128, 1], F32, tag="pp")
                nc.tensor.matmul(bq_ps[:], lhsT=c_wqT4[:], rhs=shift_c[:])
                bq_f = small.tile([128, 1], F32)
                nc.vector.tensor_tensor(
                    out=bq_f[:], in0=bq_ps[:], in1=c_bq4[:], op=mybir.AluOpType.add
                )
                bk_ps = pp.tile([128, 1], F32, tag="pp")
                nc.tensor.matmul(bk_ps[:], lhsT=c_wkT4[:], rhs=shift_c[:])
                bk_f = small.tile([128, 1], F32)
                nc.vector.tensor_tensor(
                    out=bk_f[:], in0=bk_ps[:], in1=c_bk4[:], op=mybir.AluOpType.add
                )
                vs_ps = pp.tile([DH, 1], F32, tag="pp")
                nc.tensor.matmul(vs_ps[:], lhsT=c_wvT[:], rhs=shift_c[:])
                vsum = small.tile([DH, 1], F32)
                nc.vector.tensor_copy(out=vsum[:], in_=vs_ps[:])
                # addvec row: pwaug[0, :] = vsum^T pwTa
                av_ps = pp.tile([1, C], F32, tag="pp")
                nc.tensor.matmul(av_ps[:], lhsT=vsum[:], rhs=c_pwTa[:])
                nc.vector.tensor_copy(out=pwaug[0:1, :], in_=av_ps[:])

            with (
                tc.tile_pool(name="sqpool", bufs=3, space="PSUM") as sqpool,
                tc.tile_pool(name="accp", bufs=1, space="PSUM") as accp,
                tc.tile_pool(name="projp", bufs=1, space="PSUM") as projp,
            ):
                # v^T ones column (tile written chunk-wise inside sc==0 loop)
                v_t = vtpool.tile([128, 32, DH + 1], F32R)
                ones_f = consts.tile([128, 1], F32)
                nc.vector.memset(ones_f[:], 1.0)
                nc.vector.tensor_copy(
                    out=v_t[:, :, 0], in_=ones_f[:, 0:1].to_broadcast([128, 32])
                )
                q_sb = qkpool.tile([128, HW], F32R, tag="q")
                k_sb = qkpool.tile([128, HW], F32R, tag="k")

                # ---------------- attention + epilogue ----------------
                # per s-chunk (512 cols): 8 quads of 4 t-blocks; S^T
                # row-packed 4-up (K=128 active rows -> HAM stays warm);
                # exp in [128,1024] ops; AV accumulates [denom; U] in one
                # PSUM bank.  q/k/v_t production is interleaved into the
                # loops right before first use so the exp stream starts as
                # early as possible and the builds hide in PE/ACT slack.
                ident = mybir.ActivationFunctionType.Identity
                def build_q(j):
                    qp = sqpool.tile([128, 1024], F32, tag="sq")
                    nc.tensor.matmul(
                        qp[:, 0:512], lhsT=wq_f[:], rhs=Xr[:, 512 * j : 512 * (j + 1)]
                    )
                    nc.scalar.activation(
                        out=q_sb[:, 512 * j : 512 * (j + 1)], in_=qp[:, 0:512],
                        func=ident, bias=bq_f[:], scale=1.0,
                    )

                build_q(0)
                for sc in range(HW // 512):
                    so = 512 * sc
                    acc = accp.tile([DH + 1, 512], F32)
                    prev = None
                    for tq in range(8):
                        if sc == 0:
                            kp = sqpool.tile([128, 1024], F32, tag="sq")
                            nc.tensor.matmul(
                                kp[:, 0:512], lhsT=wk_f[:],
                                rhs=Xr[:, 512 * tq : 512 * (tq + 1)],
                            )
                            nc.scalar.activation(
                                out=k_sb[:, 512 * tq : 512 * (tq + 1)],
                                in_=kp[:, 0:512],
                                func=ident, bias=bk_f[:], scale=1.0,
                            )
                        if tq == 4 and sc < 7:
                            build_q(sc + 1)
                        SA = sqpool.tile([128, 1024], F32, tag="sq")
                        SB = sqpool.tile([128, 1024], F32, tag="sq")
                        for i in range(4):
                            dst = (SA if i < 2 else SB)[:, 512 * (i % 2) : 512 * (i % 2 + 1)]
                            nc.tensor.matmul(
                                dst,
                                lhsT=k_sb[32 * i : 32 * (i + 1),
                                          128 * (4 * tq + i) : 128 * (4 * tq + i + 1)],
                                rhs=q_sb[32 * i : 32 * (i + 1), so : so + 512],
                                tile_position=(32 * i, 0),
                            )
                        if sc == 0:
                            for i in range(4):
                                tb = 4 * tq + i
                                vp = projp.tile([128, DH], F32, tag="pj")
                                nc.tensor.matmul(
                                    vp[:], lhsT=Xr[:, 128 * tb : 128 * (tb + 1)],
                                    rhs=wv_f[:],
                                )
                                nc.vector.tensor_copy(
                                    out=v_t[:, tb, 1 : DH + 1], in_=vp[:]
                                )
                        if prev is not None:
                            pA, pB, ptq = prev
                            for i in range(4):
                                src = (pA if i < 2 else pB)[:, 512 * (i % 2) : 512 * (i % 2 + 1)]
                                nc.tensor.matmul(
                                    acc[:],
                                    lhsT=v_t[:, 4 * ptq + i, :],
                                    rhs=src,
                                    start=(ptq == 0 and i == 0),
                                    stop=False,
                                )
                        EA = epool.tile([128, 1024], F32R, tag="E")
                        nc.scalar.activation(
                            out=EA[:], in_=SA[:],
                            func=mybir.ActivationFunctionType.Exp,
                            scale=float(SCALE),
                        )
                        EB = epool.tile([128, 1024], F32R, tag="E")
                        nc.scalar.activation(
                            out=EB[:], in_=SB[:],
                            func=mybir.ActivationFunctionType.Exp,
                            scale=float(SCALE),
                        )
                        prev = (EA, EB, tq)
                    pA, pB, ptq = prev
                    for i in range(4):
                        src = (pA if i < 2 else pB)[:, 512 * (i % 2) : 512 * (i % 2 + 1)]
                        nc.tensor.matmul(
                            acc[:],
                            lhsT=v_t[:, 4 * ptq + i, :],
                            rhs=src,
                            start=False,
                            stop=(i == 3),
                        )

                    # epilogue: U -> proj -> normalize -> store
                    U = upool.tile([DH + 1, 512], F32)
                    nc.vector.tensor_copy(out=U[:], in_=acc[:])
                    recip = upool.tile([1, 512], F32, tag="recip")
                    nc.vector.reciprocal(out=recip[:], in_=U[0:1, :])
                    bc = bcpool.tile([C, 512], F32)
                    nc.gpsimd.partition_broadcast(bc[:], recip[:], channels=C)
                    pj = projp.tile([C, 512], F32, tag="pj")
                    nc.tensor.matmul(pj[:], lhsT=pwaug[:], rhs=U[:])
                    out_sb = opool.tile([C, 512], F32)
                    nc.vector.tensor_tensor(
                        out=out_sb[:], in0=pj[:], in1=bc[:], op=mybir.AluOpType.mult
                    )
                    nc.sync.dma_start(out=out_d[:, so : so + 512], in_=out_sb[:])

    nc.compile()
    return nc


_NC_CACHE = None


def _get_program():
    global _NC_CACHE
    if _NC_CACHE is None:
        _NC_CACHE = build_program()
    return _NC_CACHE


def kernel(x, norm_w, norm_b, qkv_w, qkv_b, proj_w, proj_b):
    x = np.asarray(x, np.float32)
    norm_w = np.asarray(norm_w, np.float32)
    norm_b = np.asarray(norm_b, np.float32)
    qkv_w = np.asarray(qkv_w, np.float32)
    qkv_b = np.asarray(qkv_b, np.float32)
    proj_w = np.asarray(proj_w, np.float32)
    proj_b = np.asarray(proj_b, np.float32)

    nc = _get_program()

    gs = np.zeros((C, NG), np.float32)
    gs[np.arange(C), np.arange(C) // GS] = 1.0 / GS
    g2 = np.zeros((NG, C), np.float32)
    g2[np.arange(C) // GS, np.arange(C)] = 1.0

    in_maps = []
    for ci in range(N_CORES):
        b, h = ci // NH, ci % NH
        sl = slice(DH * h, DH * (h + 1))
        wqT = qkv_w[sl, :].T
        wkT = qkv_w[C:][sl, :].T
        cpk = np.concatenate(
            [
                np.tile(wqT, (1, 4)),
                np.tile(wkT, (1, 4)),
                qkv_w[2 * C:][sl, :].T,
                np.tile(qkv_b[sl].reshape(DH, 1), (4, 1)),
                np.tile(qkv_b[C:][sl].reshape(DH, 1), (4, 1)),
                gs,
                norm_w.reshape(C, 1),
                norm_b.reshape(C, 1),
            ],
            axis=1,
        )
        in_maps.append(
            {
                "xs": np.ascontiguousarray(x[b].reshape(C, HW)),
                "cpk": np.ascontiguousarray(cpk),
                "pwTa": np.ascontiguousarray(proj_w[:, sl].T),
                "g2": g2,
            }
        )

    res = run_bass_kernel_spmd(nc, in_maps, core_ids=list(range(N_CORES)))

    # unshard: sum per-head partials, add residual + proj bias + v-bias term
    base = proj_b + proj_w @ qkv_b[2 * C :]
    out = np.empty((B, C, HW), np.float32)
    for b in range(B):
        acc = np.zeros((C, HW), np.float32)
        for h in range(NH):
            acc += res.results[b * NH + h]["out_p"]
        out[b] = acc + x[b].reshape(C, HW) + base[:, None]
    return out.reshape(B, C, 64, 64)
